# revision 20
# baseline (speedup 1.0000x reference)
"""Trainium2 Bass kernel for the BKT (multi-HMM knowledge tracing) forward model.

Strategy: data-parallel over students (1024 students / 8 cores = 128 per core,
one SBUF partition per student). The T=500 time recursion runs locally per core.

Per-core algebra per step t:
    c      = A[kc[:,t]]                          [128,100]  (gathered, rows sum to 1)
    strm   = [w5_s0 | w5_s1] per (student,t)     [128,10]   (host-precomputed stream)
             where w5_s = [M4[s], M4[2+s], L4[s], L4[2+s], 0],
             M4 = (A @ log_t)[kc] + OLL(s') and L4 = log_obs[problem]
    a2_s   = sum_k c * alpha_s     (fused scalar_tensor_tensor accum; byproduct
                                    u_s = c * alpha_s is kept)
    e_s    = exp(strm_s + a2_s)                  (activation with bias=a2_s)
    ps5    = e_0 + e_1 = [se0,se1,po0,po1,q]
    lg5    = ln(ps5): a3 = lg5[:,0:2], log_py = lg5[:,2:4] - lg5[:,4:5]
    v_s    = alpha_s - u_s  (= alpha*(1-c), off the critical chain)
    alpha_s' = c * a3_s + v_s                    (one fused stt per s)

Only the A rows (indexed by kc, 512B each) are gathered on device; the small
per-step observation data (10 f32) is a pure function of the inputs and is
streamed in dense [128, T, 10] layout (loaded once into SBUF). Gathers are
spread across all 4 SWDGE queues with deep prefetch so descriptor generation
and DMA drain hide under the vector-engine recursion.

The predicted-output normalizer q = sum_o po_o collapses to e^{a2_0}+e^{a2_1}
because log_obs is normalized over o; sum_k c = 1 because A rows are a softmax.
"""

import os
from contextlib import ExitStack

import numpy as np

N_PROBLEMS = 10000
N_KCS = 100
BATCH = 1024
T_FULL = 500
N_CORES = 8
BL = BATCH // N_CORES  # 128 students per core

_CHUNK = 8  # time steps per gather slab (8*128 = 1024 = SWDGE ring capacity)
_PREFETCH = 5  # chunks of gather issued ahead of consumption
_NQ = 4  # SWDGE queues used round-robin
_BF16 = False  # keep alpha/c/u/v state in bf16 (DVE 2x mode); exp/ln stay fp32


def _log_softmax(x, axis):
    x = x.astype(np.float32)
    m = x.max(axis=axis, keepdims=True)
    e = np.exp(x - m)
    return (x - m) - np.log(e.sum(axis=axis, keepdims=True))


def _wrap_idx(flat):
    """dma_gather index layout: flat index i lives at partition i%16, col i//16,
    replicated across the 8 gpsimd cores (16-partition groups)."""
    assert flat.size % 16 == 0
    w = flat.astype(np.int16).reshape(-1, 16).T  # [16, N/16]
    return np.tile(w, (8, 1))  # [128, N/16]


def _host_tables(A, trans_logits, obs_logits_problem, init_logits):
    import ml_dtypes
    P = A.shape[0]
    K = trans_logits.shape[0]
    sdt = ml_dtypes.bfloat16 if _BF16 else np.float32
    log_t = _log_softmax(trans_logits, axis=1)  # [K,2,2] normalized over middle axis
    G = A.astype(np.float32) @ log_t.reshape(K, 4)  # [P,4] laid out (s,s')
    L = _log_softmax(obs_logits_problem, axis=2)  # [P,2,2] normalized over outputs

    taba = np.zeros((P, 128), sdt)
    taba[:, 0:100] = A.astype(sdt)

    la0 = _log_softmax(init_logits, axis=1)  # [K,2]
    alpha0 = np.empty((BL, 2 * K), sdt)
    alpha0[:, 0:K] = la0[:, 0].astype(sdt)  # s=0 block
    alpha0[:, K:] = la0[:, 1].astype(sdt)  # s=1 block
    return taba, G, L, alpha0


def _setup_act_tables():
    """Both Exp and Ln live in the 'natural_log_exp_and_others' ACT table
    set, but the default set ordering makes bacc pick a different set for
    each, inserting a ~2.7us ACT_TABLE_LOAD per activation (2 per time
    step!). Reorder the set list so that set comes first for both bacc's
    chooser and walrus (via BASS_ACT_ROOT_JSON_PATH), collapsing the loads
    to one for the whole kernel."""
    import glob
    import json
    import tempfile

    if os.environ.get("_BKT_ACT_TABLES"):
        return
    from neuronxcc.driver.Job import Job  # pyright: ignore[reportMissingImports]
    from neuronxcc.driver.jobs.support.FindActInfo import (  # pyright: ignore[reportMissingImports]
        findActInfoFile,
    )

    src = findActInfoFile(Job.getPackageDir(), "gen3")
    d = json.load(open(src))
    d["act_func_sets"] = sorted(
        d["act_func_sets"],
        key=lambda s: s["name"] != "natural_log_exp_and_others")
    tmp = tempfile.mkdtemp(prefix="bkt_act_")
    with open(tmp + "/act_info.json", "w") as f:
        json.dump(d, f)
    for p in glob.glob(os.path.dirname(src) + "/*"):
        b = os.path.basename(p)
        if b != "act_info.json":
            os.symlink(p, tmp + "/" + b)
    os.environ["BASS_ACT_ROOT_JSON_PATH"] = tmp + "/act_info.json"
    os.environ["_BKT_ACT_TABLES"] = "1"

    import concourse.bacc as bacc_mod
    import concourse.mybir as mybir

    def tables(arch):
        return {
            e["name"]: {mybir.ActivationFunctionType.from_pwp(v)
                        for v in e["act"].keys()}
            for e in d["act_func_sets"]
        }

    bacc_mod.get_activation_tables = tables


def _get_state_op():
    """Register a custom DVE op fusing both states' update into one pass:
        out[s,k] = c[k] * a3_s + v[s,k]   over [128, 2, 100]
    via subdim paging (PageIdx selects a3_0 for page 0, a3_1 for page 1).
    Halves the on-chain state-update latency vs two per-s stt ops."""
    import concourse.dve_ops as dmod
    from concourse.dve_spec import (C0, C1, One, PageIdx, Spec, Src0, Src1,
                                    select, lower as dve_lower)
    from concourse.dve_uop import DveOpSpec

    name = "BKT_STATE_SEL"
    for op in dmod.OPS:
        if op.name == name:
            return op
    spec = Spec(body=Src0 * select(PageIdx() < One, C0, C1) + Src1)
    row = max(dmod._SUB_OPCODE_FOR_NAME.values()) + 1
    dmod._SUB_OPCODE_FOR_NAME[name] = row
    uops = dve_lower(spec, ver="v3")
    sha = DveOpSpec(name=name, opcode=row, uops=uops, rd1_en=True).sha("v3")
    op = dmod.DveOp(name, spec, subdim=True, uops_sha={"v3": sha})
    dmod.OPS.append(op)
    dmod.CUSTOM_DVE_SPECS[name] = spec
    return op


def _emit_program(T, Tc):
    import concourse.mybir as mybir
    import concourse.tile as tile
    from concourse import bacc

    _setup_act_tables()
    state_op = _get_state_op()

    f32 = mybir.dt.float32
    sdt = mybir.dt.bfloat16 if _BF16 else f32
    i16 = mybir.dt.int16
    Alu = mybir.AluOpType
    Act = mybir.ActivationFunctionType
    K = N_KCS

    nc = bacc.Bacc("TRN2", target_bir_lowering=False, debug=False,
                   num_swdge_queues=_NQ)

    Th = min(2 * Tc, T)  # head steps staged in separate dense tensors
    taba = nc.dram_tensor("taba", [N_PROBLEMS, 128], sdt, kind="ExternalInput")
    strm = nc.dram_tensor("strm", [BL, T * 10], f32, kind="ExternalInput")
    kcw = nc.dram_tensor("kcw", [128, T * 8], i16, kind="ExternalInput")
    kcwh = nc.dram_tensor("kcwh", [128, Th * 8], i16, kind="ExternalInput")
    strmh = nc.dram_tensor("strmh", [BL, Th * 10], f32, kind="ExternalInput")
    alpha0 = nc.dram_tensor("alpha0", [BL, 2 * K], sdt, kind="ExternalInput")
    out = nc.dram_tensor("out", [BL, T * 2], f32, kind="ExternalOutput")

    # SWDGE descriptor ring fits 1024 descriptors; each gathered row is one
    # descriptor, so cap each dma_gather call at 1024 indices (8 steps).
    assert Tc * 128 <= 1024
    chunks = []  # (t0, tcn)
    t0 = 0
    while t0 < T:
        chunks.append((t0, min(Tc, T - t0)))
        t0 += Tc
    n_chunks = len(chunks)

    from concourse import library_config

    with ExitStack() as ctx:
        tc = ctx.enter_context(tile.TileContext(nc))
        nc.gpsimd.load_library(library_config.mlp)
        idx_pool = ctx.enter_context(tc.tile_pool(name="idx", bufs=1))
        slab_pool = ctx.enter_context(
            tc.tile_pool(name="slabs", bufs=_PREFETCH + 1))
        state_pool = ctx.enter_context(tc.tile_pool(name="state", bufs=2))
        small_pool = ctx.enter_context(tc.tile_pool(name="small", bufs=4))
        u_pool = ctx.enter_context(tc.tile_pool(name="u", bufs=2))
        out_pool = ctx.enter_context(tc.tile_pool(name="outb", bufs=1))

        # head copies land first (small dense transfers) so chunk 0/1 and the
        # first steps don't wait on the full-T kcw/strm loads
        kcwh_t = idx_pool.tile([128, Th * 8], i16, tag="kcwh")
        nc.sync.dma_start(kcwh_t[:], kcwh.ap())
        strmh_t = idx_pool.tile([128, Th * 10], f32, tag="strmh")
        nc.sync.dma_start(strmh_t[:], strmh.ap())
        alpha = state_pool.tile([128, 2 * K], sdt, tag="alpha")
        nc.sync.dma_start(alpha[:], alpha0.ap())

        kcw_t = idx_pool.tile([128, T * 8], i16, tag="kcw")
        nc.sync.dma_start(kcw_t[:], kcw.ap())
        strm_t = idx_pool.tile([128, T * 10], f32, tag="strm")
        nc.sync.dma_start(strm_t[:], strm.ap())

        outbuf = out_pool.tile([128, T * 2], f32)
        # per-step ln() results land here: [se0, se1, po0, po1, q] per t
        lgbuf = out_pool.tile([128, T * 5], f32)

        slabsA = [None] * n_chunks
        ni_regs = {}  # distinct chunk sizes get one register each
        for tcn in sorted({c[1] for c in chunks}):
            r = nc.gpsimd.alloc_register(f"ni{tcn}")
            nc.gpsimd.reg_mov(r, tcn * 128)
            ni_regs[tcn] = r

        def issue_gather(n):
            t0, tcn = chunks[n]
            ni = ni_regs[tcn]
            sa = slab_pool.tile([128, Tc, 128], sdt, tag="slabA")
            # head chunks read their indices from the fast head copy
            if t0 + tcn <= Th:
                idx = kcwh_t[:, t0 * 8:(t0 + tcn) * 8]
            else:
                idx = kcw_t[:, t0 * 8:(t0 + tcn) * 8]
            nc.gpsimd.dma_gather(
                sa[:, 0:tcn, :], taba.ap(), idx,
                num_idxs=tcn * 128, num_idxs_reg=ni, elem_size=128,
                queue_num=n % _NQ,
            )
            slabsA[n] = sa

        for n in range(min(_PREFETCH, n_chunks)):
            issue_gather(n)

        def c_ap(t):
            return slabsA[t // Tc][:, t % Tc, 0:K]

        def emit_a2(t, alpha_t):
            """a2_s = sum_k c * alpha_s, fused via scalar_tensor_tensor accum.
            alpha is blocked [128, (s k)]; slices pick each s. Returns
            (a2, u2) where u2 = c * alpha is reused for the state update."""
            a2 = small_pool.tile([128, 2], f32, tag="a2")
            u2 = u_pool.tile([128, 2 * K], sdt, tag="u2")
            for s in range(2):
                nc.vector.scalar_tensor_tensor(
                    out=u2[:, s * K:(s + 1) * K], in0=c_ap(t), scalar=0.0,
                    in1=alpha_t[:, s * K:(s + 1) * K],
                    op0=Alu.bypass, op1=Alu.mult,
                    accum_out=a2[:, s:s + 1],
                )
            return a2, u2

        # prologue: a2 for t=0
        a2, u2 = emit_a2(0, alpha)

        for n in range(n_chunks):
            if n + _PREFETCH < n_chunks:
                issue_gather(n + _PREFETCH)
            for j in range(chunks[n][1]):
                t = chunks[n][0] + j
                # e_s = exp(strm_s + a2_s); e_0 only waits on a2_0
                st = strmh_t if t < Th else strm_t
                e10 = small_pool.tile([128, 10], f32, tag="e10")
                for s in range(2):
                    nc.scalar.activation(
                        e10[:, 5 * s:5 * s + 5],
                        st[:, 10 * t + 5 * s:10 * t + 5 * s + 5],
                        Act.Exp, bias=a2[:, s:s + 1],
                    )
                # v = alpha*(1-c), off the Ln chain (runs under the Exps)
                v = u_pool.tile([128, 2 * K], sdt, tag="v")
                nc.vector.tensor_tensor(
                    out=v[:], in0=alpha[:], in1=u2[:], op=Alu.subtract,
                )
                ps5 = small_pool.tile([128, 5], f32, tag="ps5")
                nc.vector.tensor_tensor(
                    out=ps5[:], in0=e10[:, 0:5], in1=e10[:, 5:10], op=Alu.add,
                )
                lg5 = lgbuf[:, 5 * t:5 * t + 5]
                nc.scalar.activation(lg5, ps5[:], Act.Ln)
                # fused state update, both states in one custom DVE op:
                # alpha'[s,k] = c[k]*a3_s + v[s,k]
                alpha_new = state_pool.tile([128, 2 * K], sdt, tag="alpha")
                nc.vector._custom_dve(
                    state_op,
                    out=alpha_new[:],
                    in0=c_ap(t).unsqueeze(1).broadcast_to([128, 2, K]),
                    in1=v[:],
                    s0=lgbuf[:, 5 * t:5 * t + 1],
                    s1=lgbuf[:, 5 * t + 1:5 * t + 2],
                )
                last = t + 1 >= T
                if not last:
                    a2n = small_pool.tile([128, 2], f32, tag="a2")
                    u2n = u_pool.tile([128, 2 * K], sdt, tag="u2")
                    for s in range(2):
                        nc.vector.scalar_tensor_tensor(
                            out=u2n[:, s * K:(s + 1) * K], in0=c_ap(t + 1),
                            scalar=0.0,
                            in1=alpha_new[:, s * K:(s + 1) * K],
                            op0=Alu.bypass, op1=Alu.mult,
                            accum_out=a2n[:, s:s + 1],
                        )
                    a2, u2 = a2n, u2n
                alpha = alpha_new

        # normalize all outputs at once: log_py[t, o] = lpo[t, o] - lq[t]
        lg3 = lgbuf[:].rearrange("p (t f) -> p t f", f=5)
        nc.vector.tensor_tensor(
            out=outbuf[:].rearrange("p (t o) -> p t o", o=2),
            in0=lg3[:, :, 2:4],
            in1=lg3[:, :, 4:5].broadcast_to([128, T, 2]),
            op=Alu.subtract,
        )
        nc.sync.dma_start(out.ap(), outbuf[:])

    nc.compile()
    return nc


def _prep_inputs(corr, kc, problem, A, trans_logits, obs_logits_problem, init_logits, T):
    corr = np.asarray(corr).astype(np.int64)
    kc = np.asarray(kc).astype(np.int64)
    problem = np.asarray(problem).astype(np.int64)
    taba, G, L, alpha0 = _host_tables(
        np.asarray(A), np.asarray(trans_logits),
        np.asarray(obs_logits_problem), np.asarray(init_logits))

    in_maps = []
    for i in range(N_CORES):
        sl = slice(i * BL, (i + 1) * BL)
        kc_l = kc[sl, :T]  # [128, T]
        pp_l = problem[sl, :T]
        cr_l = corr[sl, :T]
        # dense per-(student,step) stream, grouped by state s:
        # strm[.., 5s:5s+5] = [M4[s], M4[2+s], L4[s], L4[2+s], 0]
        Gk = G[kc_l]  # [128,T,4]
        Lp = L[pp_l]  # [128,T,2,2]
        OLL = np.take_along_axis(
            Lp, cr_l[:, :, None, None], axis=3)[:, :, :, 0]  # [128,T,2]
        M4 = (Gk.reshape(BL, T, 2, 2) + OLL[:, :, None, :]).reshape(BL, T, 4)
        L4 = Lp.transpose(0, 1, 3, 2).reshape(BL, T, 4)  # L4[2o+s] = L[s,o]
        Z = np.zeros((BL, T), np.float32)
        strm = np.stack(
            [M4[..., 0], M4[..., 2], L4[..., 0], L4[..., 2], Z,
             M4[..., 1], M4[..., 3], L4[..., 1], L4[..., 3], Z],
            axis=2).astype(np.float32).reshape(BL, T * 10)
        # gather flat order: i = j*128 + p  ->  idx = kc_l[p, j]
        kcw = _wrap_idx(kc_l.T.ravel())
        Th = min(2 * _CHUNK, T)
        in_maps.append({
            "taba": taba, "strm": strm, "kcw": kcw, "alpha0": alpha0,
            "kcwh": np.ascontiguousarray(kcw[:, 0:Th * 8]),
            "strmh": np.ascontiguousarray(strm[:, 0:Th * 10]),
        })
    return in_maps


def kernel(corr, kc, problem, A, trans_logits, obs_logits_problem, init_logits,
           _T=None, _trace=False):
    T = _T or T_FULL
    nc = _emit_program(T, min(_CHUNK, T))
    in_maps = _prep_inputs(corr, kc, problem, A, trans_logits,
                           obs_logits_problem, init_logits, T)

    from concourse.bass_utils import run_bass_kernel_spmd
    res = run_bass_kernel_spmd(nc, in_maps, core_ids=list(range(N_CORES)),
                               trace=_trace)
    outs = [r["out"].reshape(BL, T, 2) for r in res.results]
    full = np.concatenate(outs, axis=0).astype(np.float32)
    kernel.last_results = res
    return full


if __name__ == "__main__":
    pass


# revision 22
# speedup vs baseline: 1.0284x; 1.0284x over previous
"""Trainium2 Bass kernel for the BKT (multi-HMM knowledge tracing) forward model.

Strategy: data-parallel over students (1024 students / 8 cores = 128 per core,
one SBUF partition per student). The T=500 time recursion runs locally per core.

Per-core algebra per step t:
    c      = A[kc[:,t]]                          [128,100]  (gathered, rows sum to 1)
    strm   = [w5_s0 | w5_s1] per (student,t)     [128,10]   (host-precomputed stream)
             where w5_s = [M4[s], M4[2+s], L4[s], L4[2+s], 0],
             M4 = (A @ log_t)[kc] + OLL(s') and L4 = log_obs[problem]
    a2_s   = sum_k c * alpha_s     (fused scalar_tensor_tensor accum; byproduct
                                    u_s = c * alpha_s is kept)
    e_s    = exp(strm_s + a2_s)                  (activation with bias=a2_s)
    ps5    = e_0 + e_1 = [se0,se1,po0,po1,q]
    lg5    = ln(ps5): a3 = lg5[:,0:2], log_py = lg5[:,2:4] - lg5[:,4:5]
    v_s    = alpha_s - u_s  (= alpha*(1-c), off the critical chain)
    alpha_s' = c * a3_s + v_s                    (one fused stt per s)

Only the A rows (indexed by kc, 512B each) are gathered on device; the small
per-step observation data (10 f32) is a pure function of the inputs and is
streamed in dense [128, T, 10] layout (loaded once into SBUF). Gathers are
spread across all 4 SWDGE queues with deep prefetch so descriptor generation
and DMA drain hide under the vector-engine recursion.

The predicted-output normalizer q = sum_o po_o collapses to e^{a2_0}+e^{a2_1}
because log_obs is normalized over o; sum_k c = 1 because A rows are a softmax.
"""

import os
from contextlib import ExitStack

import numpy as np

N_PROBLEMS = 10000
N_KCS = 100
BATCH = 1024
T_FULL = 500
N_CORES = 8
BL = BATCH // N_CORES  # 128 students per core

_CHUNK = 8  # time steps per gather slab (8*128 = 1024 = SWDGE ring capacity)
_PREFETCH = 5  # chunks of gather issued ahead of consumption
_NQ = 4  # SWDGE queues used round-robin
_BF16 = False  # keep alpha/c/u/v state in bf16 (DVE 2x mode); exp/ln stay fp32


def _log_softmax(x, axis):
    x = x.astype(np.float32)
    m = x.max(axis=axis, keepdims=True)
    e = np.exp(x - m)
    return (x - m) - np.log(e.sum(axis=axis, keepdims=True))


def _wrap_idx(flat):
    """dma_gather index layout: flat index i lives at partition i%16, col i//16,
    replicated across the 8 gpsimd cores (16-partition groups)."""
    assert flat.size % 16 == 0
    w = flat.astype(np.int16).reshape(-1, 16).T  # [16, N/16]
    return np.tile(w, (8, 1))  # [128, N/16]


def _host_tables(A, trans_logits, obs_logits_problem, init_logits):
    import ml_dtypes
    P = A.shape[0]
    K = trans_logits.shape[0]
    sdt = ml_dtypes.bfloat16 if _BF16 else np.float32
    log_t = _log_softmax(trans_logits, axis=1)  # [K,2,2] normalized over middle axis
    G = A.astype(np.float32) @ log_t.reshape(K, 4)  # [P,4] laid out (s,s')
    L = _log_softmax(obs_logits_problem, axis=2)  # [P,2,2] normalized over outputs

    taba = np.zeros((P, 128), sdt)
    taba[:, 0:100] = A.astype(sdt)

    la0 = _log_softmax(init_logits, axis=1)  # [K,2]
    alpha0 = np.empty((BL, 2 * K), sdt)
    alpha0[:, 0:K] = la0[:, 0].astype(sdt)  # s=0 block
    alpha0[:, K:] = la0[:, 1].astype(sdt)  # s=1 block
    return taba, G, L, alpha0


def _setup_act_tables():
    """Both Exp and Ln live in the 'natural_log_exp_and_others' ACT table
    set, but the default set ordering makes bacc pick a different set for
    each, inserting a ~2.7us ACT_TABLE_LOAD per activation (2 per time
    step!). Reorder the set list so that set comes first for both bacc's
    chooser and walrus (via BASS_ACT_ROOT_JSON_PATH), collapsing the loads
    to one for the whole kernel."""
    import glob
    import json
    import tempfile

    if os.environ.get("_BKT_ACT_TABLES"):
        return
    from neuronxcc.driver.Job import Job  # pyright: ignore[reportMissingImports]
    from neuronxcc.driver.jobs.support.FindActInfo import (  # pyright: ignore[reportMissingImports]
        findActInfoFile,
    )

    src = findActInfoFile(Job.getPackageDir(), "gen3")
    d = json.load(open(src))
    d["act_func_sets"] = sorted(
        d["act_func_sets"],
        key=lambda s: s["name"] != "natural_log_exp_and_others")
    tmp = tempfile.mkdtemp(prefix="bkt_act_")
    with open(tmp + "/act_info.json", "w") as f:
        json.dump(d, f)
    for p in glob.glob(os.path.dirname(src) + "/*"):
        b = os.path.basename(p)
        if b != "act_info.json":
            os.symlink(p, tmp + "/" + b)
    os.environ["BASS_ACT_ROOT_JSON_PATH"] = tmp + "/act_info.json"
    os.environ["_BKT_ACT_TABLES"] = "1"

    import concourse.bacc as bacc_mod
    import concourse.mybir as mybir

    def tables(arch):
        return {
            e["name"]: {mybir.ActivationFunctionType.from_pwp(v)
                        for v in e["act"].keys()}
            for e in d["act_func_sets"]
        }

    bacc_mod.get_activation_tables = tables


def _emit_program(T, Tc):
    import concourse.mybir as mybir
    import concourse.tile as tile
    from concourse import bacc

    _setup_act_tables()

    f32 = mybir.dt.float32
    sdt = mybir.dt.bfloat16 if _BF16 else f32
    i16 = mybir.dt.int16
    Alu = mybir.AluOpType
    Act = mybir.ActivationFunctionType
    K = N_KCS

    nc = bacc.Bacc("TRN2", target_bir_lowering=False, debug=False,
                   num_swdge_queues=_NQ)

    Th = min(2 * Tc, T)  # head steps staged in separate dense tensors
    taba = nc.dram_tensor("taba", [N_PROBLEMS, 128], sdt, kind="ExternalInput")
    strm = nc.dram_tensor("strm", [BL, T * 10], f32, kind="ExternalInput")
    kcw = nc.dram_tensor("kcw", [128, T * 8], i16, kind="ExternalInput")
    kcwh = nc.dram_tensor("kcwh", [128, Th * 8], i16, kind="ExternalInput")
    strmh = nc.dram_tensor("strmh", [BL, Th * 10], f32, kind="ExternalInput")
    alpha0 = nc.dram_tensor("alpha0", [BL, 2 * K], sdt, kind="ExternalInput")
    out = nc.dram_tensor("out", [BL, T * 2], f32, kind="ExternalOutput")

    # SWDGE descriptor ring fits 1024 descriptors; each gathered row is one
    # descriptor, so cap each dma_gather call at 1024 indices (8 steps).
    assert Tc * 128 <= 1024
    chunks = []  # (t0, tcn)
    t0 = 0
    while t0 < T:
        chunks.append((t0, min(Tc, T - t0)))
        t0 += Tc
    n_chunks = len(chunks)

    from concourse import library_config

    with ExitStack() as ctx:
        tc = ctx.enter_context(tile.TileContext(nc))
        nc.gpsimd.load_library(library_config.mlp)
        idx_pool = ctx.enter_context(tc.tile_pool(name="idx", bufs=1))
        slab_pool = ctx.enter_context(
            tc.tile_pool(name="slabs", bufs=_PREFETCH + 1))
        state_pool = ctx.enter_context(tc.tile_pool(name="state", bufs=2))
        small_pool = ctx.enter_context(tc.tile_pool(name="small", bufs=4))
        u_pool = ctx.enter_context(tc.tile_pool(name="u", bufs=2))
        out_pool = ctx.enter_context(tc.tile_pool(name="outb", bufs=1))

        # head copies land first (small dense transfers) so chunk 0/1 and the
        # first steps don't wait on the full-T kcw/strm loads
        kcwh_t = idx_pool.tile([128, Th * 8], i16, tag="kcwh")
        nc.sync.dma_start(kcwh_t[:], kcwh.ap())
        strmh_t = idx_pool.tile([128, Th * 10], f32, tag="strmh")
        nc.sync.dma_start(strmh_t[:], strmh.ap())
        alpha = state_pool.tile([128, 2 * K], sdt, tag="alpha")
        nc.sync.dma_start(alpha[:], alpha0.ap())

        kcw_t = idx_pool.tile([128, T * 8], i16, tag="kcw")
        nc.sync.dma_start(kcw_t[:], kcw.ap())
        strm_t = idx_pool.tile([128, T * 10], f32, tag="strm")
        nc.sync.dma_start(strm_t[:], strm.ap())

        outbuf = out_pool.tile([128, T * 2], f32)
        # per-step ln() results land here: [se0, se1, po0, po1, q] per t
        lgbuf = out_pool.tile([128, T * 5], f32)

        slabsA = [None] * n_chunks
        ni_regs = {}  # distinct chunk sizes get one register each
        for tcn in sorted({c[1] for c in chunks}):
            r = nc.gpsimd.alloc_register(f"ni{tcn}")
            nc.gpsimd.reg_mov(r, tcn * 128)
            ni_regs[tcn] = r

        def issue_gather(n):
            t0, tcn = chunks[n]
            ni = ni_regs[tcn]
            sa = slab_pool.tile([128, Tc, 128], sdt, tag="slabA")
            # head chunks read their indices from the fast head copy
            if t0 + tcn <= Th:
                idx = kcwh_t[:, t0 * 8:(t0 + tcn) * 8]
            else:
                idx = kcw_t[:, t0 * 8:(t0 + tcn) * 8]
            nc.gpsimd.dma_gather(
                sa[:, 0:tcn, :], taba.ap(), idx,
                num_idxs=tcn * 128, num_idxs_reg=ni, elem_size=128,
                queue_num=n % _NQ,
            )
            slabsA[n] = sa

        for n in range(min(_PREFETCH, n_chunks)):
            issue_gather(n)

        def c_ap(t):
            return slabsA[t // Tc][:, t % Tc, 0:K]

        def emit_a2(t, alpha_t):
            """a2_s = sum_k c * alpha_s, fused via scalar_tensor_tensor accum.
            alpha is blocked [128, (s k)]; slices pick each s. Returns
            (a2, u2) where u2 = c * alpha is reused for the state update."""
            a2 = small_pool.tile([128, 2], f32, tag="a2")
            u2 = u_pool.tile([128, 2 * K], sdt, tag="u2")
            for s in range(2):
                nc.vector.scalar_tensor_tensor(
                    out=u2[:, s * K:(s + 1) * K], in0=c_ap(t), scalar=0.0,
                    in1=alpha_t[:, s * K:(s + 1) * K],
                    op0=Alu.bypass, op1=Alu.mult,
                    accum_out=a2[:, s:s + 1],
                )
            return a2, u2

        # prologue: a2 for t=0
        a2, u2 = emit_a2(0, alpha)

        for n in range(n_chunks):
            if n + _PREFETCH < n_chunks:
                issue_gather(n + _PREFETCH)
            for j in range(chunks[n][1]):
                t = chunks[n][0] + j
                # e_s = exp(strm_s + a2_s); e_0 only waits on a2_0
                st = strmh_t if t < Th else strm_t
                e10 = small_pool.tile([128, 10], f32, tag="e10")
                for s in range(2):
                    nc.scalar.activation(
                        e10[:, 5 * s:5 * s + 5],
                        st[:, 10 * t + 5 * s:10 * t + 5 * s + 5],
                        Act.Exp, bias=a2[:, s:s + 1],
                    )
                # v = alpha*(1-c), off the Ln chain (runs under the Exps)
                v = u_pool.tile([128, 2 * K], sdt, tag="v")
                nc.vector.tensor_tensor(
                    out=v[:], in0=alpha[:], in1=u2[:], op=Alu.subtract,
                )
                ps5 = small_pool.tile([128, 5], f32, tag="ps5")
                nc.vector.tensor_tensor(
                    out=ps5[:], in0=e10[:, 0:5], in1=e10[:, 5:10], op=Alu.add,
                )
                lg5 = lgbuf[:, 5 * t:5 * t + 5]
                nc.scalar.activation(lg5, ps5[:], Act.Ln)
                # state update per s: alpha_s' = c*a3_s + v_s, then a2 for
                # s before moving to the other s so Exp_0 can start early
                alpha_new = state_pool.tile([128, 2 * K], sdt, tag="alpha")
                last = t + 1 >= T
                if not last:
                    a2n = small_pool.tile([128, 2], f32, tag="a2")
                    u2n = u_pool.tile([128, 2 * K], sdt, tag="u2")
                for s in range(2):
                    nc.vector.scalar_tensor_tensor(
                        out=alpha_new[:, s * K:(s + 1) * K], in0=c_ap(t),
                        scalar=lgbuf[:, 5 * t + s:5 * t + s + 1],
                        in1=v[:, s * K:(s + 1) * K],
                        op0=Alu.mult, op1=Alu.add,
                    )
                    if not last:
                        nc.vector.scalar_tensor_tensor(
                            out=u2n[:, s * K:(s + 1) * K], in0=c_ap(t + 1),
                            scalar=0.0,
                            in1=alpha_new[:, s * K:(s + 1) * K],
                            op0=Alu.bypass, op1=Alu.mult,
                            accum_out=a2n[:, s:s + 1],
                        )
                alpha = alpha_new
                if not last:
                    a2, u2 = a2n, u2n

        # normalize all outputs at once: log_py[t, o] = lpo[t, o] - lq[t]
        lg3 = lgbuf[:].rearrange("p (t f) -> p t f", f=5)
        nc.vector.tensor_tensor(
            out=outbuf[:].rearrange("p (t o) -> p t o", o=2),
            in0=lg3[:, :, 2:4],
            in1=lg3[:, :, 4:5].broadcast_to([128, T, 2]),
            op=Alu.subtract,
        )
        nc.sync.dma_start(out.ap(), outbuf[:])

    nc.compile()
    return nc


def _prep_inputs(corr, kc, problem, A, trans_logits, obs_logits_problem, init_logits, T):
    corr = np.asarray(corr).astype(np.int64)
    kc = np.asarray(kc).astype(np.int64)
    problem = np.asarray(problem).astype(np.int64)
    taba, G, L, alpha0 = _host_tables(
        np.asarray(A), np.asarray(trans_logits),
        np.asarray(obs_logits_problem), np.asarray(init_logits))

    in_maps = []
    for i in range(N_CORES):
        sl = slice(i * BL, (i + 1) * BL)
        kc_l = kc[sl, :T]  # [128, T]
        pp_l = problem[sl, :T]
        cr_l = corr[sl, :T]
        # dense per-(student,step) stream, grouped by state s:
        # strm[.., 5s:5s+5] = [M4[s], M4[2+s], L4[s], L4[2+s], 0]
        Gk = G[kc_l]  # [128,T,4]
        Lp = L[pp_l]  # [128,T,2,2]
        OLL = np.take_along_axis(
            Lp, cr_l[:, :, None, None], axis=3)[:, :, :, 0]  # [128,T,2]
        M4 = (Gk.reshape(BL, T, 2, 2) + OLL[:, :, None, :]).reshape(BL, T, 4)
        L4 = Lp.transpose(0, 1, 3, 2).reshape(BL, T, 4)  # L4[2o+s] = L[s,o]
        Z = np.zeros((BL, T), np.float32)
        strm = np.stack(
            [M4[..., 0], M4[..., 2], L4[..., 0], L4[..., 2], Z,
             M4[..., 1], M4[..., 3], L4[..., 1], L4[..., 3], Z],
            axis=2).astype(np.float32).reshape(BL, T * 10)
        # gather flat order: i = j*128 + p  ->  idx = kc_l[p, j]
        kcw = _wrap_idx(kc_l.T.ravel())
        Th = min(2 * _CHUNK, T)
        in_maps.append({
            "taba": taba, "strm": strm, "kcw": kcw, "alpha0": alpha0,
            "kcwh": np.ascontiguousarray(kcw[:, 0:Th * 8]),
            "strmh": np.ascontiguousarray(strm[:, 0:Th * 10]),
        })
    return in_maps


def kernel(corr, kc, problem, A, trans_logits, obs_logits_problem, init_logits,
           _T=None, _trace=False):
    T = _T or T_FULL
    nc = _emit_program(T, min(_CHUNK, T))
    in_maps = _prep_inputs(corr, kc, problem, A, trans_logits,
                           obs_logits_problem, init_logits, T)

    from concourse.bass_utils import run_bass_kernel_spmd
    res = run_bass_kernel_spmd(nc, in_maps, core_ids=list(range(N_CORES)),
                               trace=_trace)
    outs = [r["out"].reshape(BL, T, 2) for r in res.results]
    full = np.concatenate(outs, axis=0).astype(np.float32)
    kernel.last_results = res
    return full


if __name__ == "__main__":
    pass


# revision 26
# speedup vs baseline: 1.0342x; 1.0057x over previous
"""Trainium2 Bass kernel for the BKT (multi-HMM knowledge tracing) forward model.

Strategy: data-parallel over students (1024 students / 8 cores = 128 per core,
one SBUF partition per student). The T=500 time recursion runs locally per core.

Per-core algebra per step t:
    c      = A[kc[:,t]]                          [128,100]  (gathered, rows sum to 1)
    strm   = [w5_s0 | w5_s1] per (student,t)     [128,10]   (host-precomputed stream)
             where w5_s = [M4[s], M4[2+s], L4[s], L4[2+s], 0],
             M4 = (A @ log_t)[kc] + OLL(s') and L4 = log_obs[problem]
    a2_s   = sum_k c * alpha_s     (fused scalar_tensor_tensor accum; byproduct
                                    u_s = c * alpha_s is kept)
    e_s    = exp(strm_s + a2_s)                  (activation with bias=a2_s)
    ps5    = e_0 + e_1 = [se0,se1,po0,po1,q]
    lg5    = ln(ps5): a3 = lg5[:,0:2], log_py = lg5[:,2:4] - lg5[:,4:5]
    v_s    = alpha_s - u_s  (= alpha*(1-c), off the critical chain)
    alpha_s' = c * a3_s + v_s                    (one fused stt per s)

Only the A rows (indexed by kc, 512B each) are gathered on device; the small
per-step observation data (10 f32) is a pure function of the inputs and is
streamed in dense [128, T, 10] layout (loaded once into SBUF). Gathers are
spread across all 4 SWDGE queues with deep prefetch so descriptor generation
and DMA drain hide under the vector-engine recursion.

The predicted-output normalizer q = sum_o po_o collapses to e^{a2_0}+e^{a2_1}
because log_obs is normalized over o; sum_k c = 1 because A rows are a softmax.
"""

import os
from contextlib import ExitStack

import numpy as np

N_PROBLEMS = 10000
N_KCS = 100
BATCH = 1024
T_FULL = 500
N_CORES = 8
BL = BATCH // N_CORES  # 128 students per core

_CHUNK = 8  # time steps per gather slab (8*128 = 1024 = SWDGE ring capacity)
_PREFETCH = 5  # chunks of gather issued ahead of consumption
_NQ = 4  # SWDGE queues used round-robin
_BF16 = False  # keep alpha/c/u/v state in bf16 (DVE 2x mode); exp/ln stay fp32


def _log_softmax(x, axis):
    x = x.astype(np.float32)
    m = x.max(axis=axis, keepdims=True)
    e = np.exp(x - m)
    return (x - m) - np.log(e.sum(axis=axis, keepdims=True))


def _wrap_idx(flat):
    """dma_gather index layout: flat index i lives at partition i%16, col i//16,
    replicated across the 8 gpsimd cores (16-partition groups)."""
    assert flat.size % 16 == 0
    w = flat.astype(np.int16).reshape(-1, 16).T  # [16, N/16]
    return np.tile(w, (8, 1))  # [128, N/16]


def _host_tables(A, trans_logits, obs_logits_problem, init_logits):
    import ml_dtypes
    P = A.shape[0]
    K = trans_logits.shape[0]
    sdt = ml_dtypes.bfloat16 if _BF16 else np.float32
    log_t = _log_softmax(trans_logits, axis=1)  # [K,2,2] normalized over middle axis
    G = A.astype(np.float32) @ log_t.reshape(K, 4)  # [P,4] laid out (s,s')
    L = _log_softmax(obs_logits_problem, axis=2)  # [P,2,2] normalized over outputs

    taba = np.zeros((P, 128), sdt)
    taba[:, 0:100] = A.astype(sdt)

    la0 = _log_softmax(init_logits, axis=1)  # [K,2]
    alpha0 = np.empty((BL, 2 * K), sdt)
    alpha0[:, 0:K] = la0[:, 0].astype(sdt)  # s=0 block
    alpha0[:, K:] = la0[:, 1].astype(sdt)  # s=1 block
    return taba, G, L, alpha0


def _setup_act_tables():
    """Both Exp and Ln live in the 'natural_log_exp_and_others' ACT table
    set, but the default set ordering makes bacc pick a different set for
    each, inserting a ~2.7us ACT_TABLE_LOAD per activation (2 per time
    step!). Reorder the set list so that set comes first for both bacc's
    chooser and walrus (via BASS_ACT_ROOT_JSON_PATH), collapsing the loads
    to one for the whole kernel."""
    import glob
    import json
    import tempfile

    if os.environ.get("_BKT_ACT_TABLES"):
        return
    from neuronxcc.driver.Job import Job  # pyright: ignore[reportMissingImports]
    from neuronxcc.driver.jobs.support.FindActInfo import (  # pyright: ignore[reportMissingImports]
        findActInfoFile,
    )

    src = findActInfoFile(Job.getPackageDir(), "gen3")
    d = json.load(open(src))
    d["act_func_sets"] = sorted(
        d["act_func_sets"],
        key=lambda s: s["name"] != "natural_log_exp_and_others")
    tmp = tempfile.mkdtemp(prefix="bkt_act_")
    with open(tmp + "/act_info.json", "w") as f:
        json.dump(d, f)
    for p in glob.glob(os.path.dirname(src) + "/*"):
        b = os.path.basename(p)
        if b != "act_info.json":
            os.symlink(p, tmp + "/" + b)
    os.environ["BASS_ACT_ROOT_JSON_PATH"] = tmp + "/act_info.json"
    os.environ["_BKT_ACT_TABLES"] = "1"

    import concourse.bacc as bacc_mod
    import concourse.mybir as mybir

    def tables(arch):
        return {
            e["name"]: {mybir.ActivationFunctionType.from_pwp(v)
                        for v in e["act"].keys()}
            for e in d["act_func_sets"]
        }

    bacc_mod.get_activation_tables = tables


def _emit_program(T, Tc):
    import concourse.mybir as mybir
    import concourse.tile as tile
    from concourse import bacc

    _setup_act_tables()

    f32 = mybir.dt.float32
    sdt = mybir.dt.bfloat16 if _BF16 else f32
    i16 = mybir.dt.int16
    Alu = mybir.AluOpType
    Act = mybir.ActivationFunctionType
    K = N_KCS

    nc = bacc.Bacc("TRN2", target_bir_lowering=False, debug=False,
                   num_swdge_queues=_NQ)

    Th = min(2 * Tc, T)  # head steps staged in separate dense tensors
    n_dense = min(_PREFETCH, (T + Tc - 1) // Tc)  # host-pregathered chunks
    taba = nc.dram_tensor("taba", [N_PROBLEMS, 128], sdt, kind="ExternalInput")
    strm = nc.dram_tensor("strm", [BL, T * 10], f32, kind="ExternalInput")
    kcw = nc.dram_tensor("kcw", [128, T * 8], i16, kind="ExternalInput")
    kcwh = nc.dram_tensor("kcwh", [128, Th * 8], i16, kind="ExternalInput")
    strmh = nc.dram_tensor("strmh", [BL, Th * 10], f32, kind="ExternalInput")
    alpha0 = nc.dram_tensor("alpha0", [BL, 2 * K], sdt, kind="ExternalInput")
    slabd = [nc.dram_tensor(f"slabd{n}",
                            [128, min(Tc, T - n * Tc) * 128], sdt,
                            kind="ExternalInput")
             for n in range(n_dense)]
    out = nc.dram_tensor("out", [BL, T * 2], f32, kind="ExternalOutput")

    # SWDGE descriptor ring fits 1024 descriptors; each gathered row is one
    # descriptor, so cap each dma_gather call at 1024 indices (8 steps).
    assert Tc * 128 <= 1024
    chunks = []  # (t0, tcn)
    t0 = 0
    while t0 < T:
        chunks.append((t0, min(Tc, T - t0)))
        t0 += Tc
    n_chunks = len(chunks)

    from concourse import library_config

    with ExitStack() as ctx:
        tc = ctx.enter_context(tile.TileContext(nc))
        nc.gpsimd.load_library(library_config.mlp)
        idx_pool = ctx.enter_context(tc.tile_pool(name="idx", bufs=1))
        slab_pool = ctx.enter_context(
            tc.tile_pool(name="slabs", bufs=_PREFETCH + 1))
        state_pool = ctx.enter_context(tc.tile_pool(name="state", bufs=2))
        small_pool = ctx.enter_context(tc.tile_pool(name="small", bufs=4))
        u_pool = ctx.enter_context(tc.tile_pool(name="u", bufs=2))
        out_pool = ctx.enter_context(tc.tile_pool(name="outb", bufs=1))

        # head copies land first (small dense transfers) so chunk 0/1 and the
        # first steps don't wait on the full-T kcw/strm loads
        kcwh_t = idx_pool.tile([128, Th * 8], i16, tag="kcwh")
        nc.sync.dma_start(kcwh_t[:], kcwh.ap())
        strmh_t = idx_pool.tile([128, Th * 10], f32, tag="strmh")
        nc.sync.dma_start(strmh_t[:], strmh.ap())
        alpha = state_pool.tile([128, 2 * K], sdt, tag="alpha")
        nc.sync.dma_start(alpha[:], alpha0.ap())

        kcw_t = idx_pool.tile([128, T * 8], i16, tag="kcw")
        nc.sync.dma_start(kcw_t[:], kcw.ap())
        strm_t = idx_pool.tile([128, T * 10], f32, tag="strm")
        nc.sync.dma_start(strm_t[:], strm.ap())

        outbuf = out_pool.tile([128, T * 2], f32)
        # per-step ln() results land here: [se0, se1, po0, po1, q] per t
        lgbuf = out_pool.tile([128, T * 5], f32)

        slabsA = [None] * n_chunks
        ni_regs = {}  # distinct chunk sizes get one register each
        for tcn in sorted({c[1] for c in chunks}):
            r = nc.gpsimd.alloc_register(f"ni{tcn}")
            nc.gpsimd.reg_mov(r, tcn * 128)
            ni_regs[tcn] = r

        def issue_gather(n):
            t0, tcn = chunks[n]
            sa = slab_pool.tile([128, Tc, 128], sdt, tag="slabA")
            if n < len(slabd):
                # first chunks come host-pregathered as dense DMAs so the
                # recursion starts without waiting for the gpsimd library
                # load + first dma_gather (~27us of startup otherwise)
                nc.sync.dma_start(sa[:, 0:tcn, :], slabd[n].ap())
            else:
                # head chunks read their indices from the fast head copy
                if t0 + tcn <= Th:
                    idx = kcwh_t[:, t0 * 8:(t0 + tcn) * 8]
                else:
                    idx = kcw_t[:, t0 * 8:(t0 + tcn) * 8]
                nc.gpsimd.dma_gather(
                    sa[:, 0:tcn, :], taba.ap(), idx,
                    num_idxs=tcn * 128, num_idxs_reg=ni_regs[tcn],
                    elem_size=128, queue_num=n % _NQ,
                )
            slabsA[n] = sa

        for n in range(min(_PREFETCH, n_chunks)):
            issue_gather(n)

        def c_ap(t):
            return slabsA[t // Tc][:, t % Tc, 0:K]

        def emit_a2(t, alpha_t):
            """a2_s = sum_k c * alpha_s, fused via scalar_tensor_tensor accum.
            alpha is blocked [128, (s k)]; slices pick each s. Returns
            (a2, u2) where u2 = c * alpha is reused for the state update."""
            a2 = small_pool.tile([128, 2], f32, tag="a2")
            u2 = u_pool.tile([128, 2 * K], sdt, tag="u2")
            for s in range(2):
                nc.vector.scalar_tensor_tensor(
                    out=u2[:, s * K:(s + 1) * K], in0=c_ap(t), scalar=0.0,
                    in1=alpha_t[:, s * K:(s + 1) * K],
                    op0=Alu.bypass, op1=Alu.mult,
                    accum_out=a2[:, s:s + 1],
                )
            return a2, u2

        # prologue: a2 for t=0
        a2, u2 = emit_a2(0, alpha)

        for n in range(n_chunks):
            if n + _PREFETCH < n_chunks:
                issue_gather(n + _PREFETCH)
            for j in range(chunks[n][1]):
                t = chunks[n][0] + j
                # e_s = exp(strm_s + a2_s); e_0 only waits on a2_0
                st = strmh_t if t < Th else strm_t
                e10 = small_pool.tile([128, 10], f32, tag="e10")
                for s in range(2):
                    nc.scalar.activation(
                        e10[:, 5 * s:5 * s + 5],
                        st[:, 10 * t + 5 * s:10 * t + 5 * s + 5],
                        Act.Exp, bias=a2[:, s:s + 1],
                    )
                # v = alpha*(1-c), off the Ln chain (runs under the Exps)
                v = u_pool.tile([128, 2 * K], sdt, tag="v")
                nc.vector.tensor_tensor(
                    out=v[:], in0=alpha[:], in1=u2[:], op=Alu.subtract,
                )
                ps5 = small_pool.tile([128, 5], f32, tag="ps5")
                nc.vector.tensor_tensor(
                    out=ps5[:], in0=e10[:, 0:5], in1=e10[:, 5:10], op=Alu.add,
                )
                lg5 = lgbuf[:, 5 * t:5 * t + 5]
                nc.scalar.activation(lg5, ps5[:], Act.Ln)
                # state update per s: alpha_s' = c*a3_s + v_s, then a2 for
                # s before moving to the other s so Exp_0 can start early
                alpha_new = state_pool.tile([128, 2 * K], sdt, tag="alpha")
                last = t + 1 >= T
                if not last:
                    a2n = small_pool.tile([128, 2], f32, tag="a2")
                    u2n = u_pool.tile([128, 2 * K], sdt, tag="u2")
                for s in range(2):
                    nc.vector.scalar_tensor_tensor(
                        out=alpha_new[:, s * K:(s + 1) * K], in0=c_ap(t),
                        scalar=lgbuf[:, 5 * t + s:5 * t + s + 1],
                        in1=v[:, s * K:(s + 1) * K],
                        op0=Alu.mult, op1=Alu.add,
                    )
                    if not last:
                        nc.vector.scalar_tensor_tensor(
                            out=u2n[:, s * K:(s + 1) * K], in0=c_ap(t + 1),
                            scalar=0.0,
                            in1=alpha_new[:, s * K:(s + 1) * K],
                            op0=Alu.bypass, op1=Alu.mult,
                            accum_out=a2n[:, s:s + 1],
                        )
                alpha = alpha_new
                if not last:
                    a2, u2 = a2n, u2n

        # normalize all outputs at once: log_py[t, o] = lpo[t, o] - lq[t]
        lg3 = lgbuf[:].rearrange("p (t f) -> p t f", f=5)
        nc.vector.tensor_tensor(
            out=outbuf[:].rearrange("p (t o) -> p t o", o=2),
            in0=lg3[:, :, 2:4],
            in1=lg3[:, :, 4:5].broadcast_to([128, T, 2]),
            op=Alu.subtract,
        )
        nc.sync.dma_start(out.ap(), outbuf[:])

    nc.compile()
    return nc


def _prep_inputs(corr, kc, problem, A, trans_logits, obs_logits_problem, init_logits, T):
    corr = np.asarray(corr).astype(np.int64)
    kc = np.asarray(kc).astype(np.int64)
    problem = np.asarray(problem).astype(np.int64)
    taba, G, L, alpha0 = _host_tables(
        np.asarray(A), np.asarray(trans_logits),
        np.asarray(obs_logits_problem), np.asarray(init_logits))

    in_maps = []
    for i in range(N_CORES):
        sl = slice(i * BL, (i + 1) * BL)
        kc_l = kc[sl, :T]  # [128, T]
        pp_l = problem[sl, :T]
        cr_l = corr[sl, :T]
        # dense per-(student,step) stream, grouped by state s:
        # strm[.., 5s:5s+5] = [M4[s], M4[2+s], L4[s], L4[2+s], 0]
        Gk = G[kc_l]  # [128,T,4]
        Lp = L[pp_l]  # [128,T,2,2]
        OLL = np.take_along_axis(
            Lp, cr_l[:, :, None, None], axis=3)[:, :, :, 0]  # [128,T,2]
        M4 = (Gk.reshape(BL, T, 2, 2) + OLL[:, :, None, :]).reshape(BL, T, 4)
        L4 = Lp.transpose(0, 1, 3, 2).reshape(BL, T, 4)  # L4[2o+s] = L[s,o]
        Z = np.zeros((BL, T), np.float32)
        strm = np.stack(
            [M4[..., 0], M4[..., 2], L4[..., 0], L4[..., 2], Z,
             M4[..., 1], M4[..., 3], L4[..., 1], L4[..., 3], Z],
            axis=2).astype(np.float32).reshape(BL, T * 10)
        # gather flat order: i = j*128 + p  ->  idx = kc_l[p, j]
        kcw = _wrap_idx(kc_l.T.ravel())
        Th = min(2 * _CHUNK, T)
        im = {
            "taba": taba, "strm": strm, "kcw": kcw, "alpha0": alpha0,
            "kcwh": np.ascontiguousarray(kcw[:, 0:Th * 8]),
            "strmh": np.ascontiguousarray(strm[:, 0:Th * 10]),
        }
        # host-pregathered A slabs for the first _PREFETCH chunks: matches
        # the dma_gather output layout slab[p, j, :] = taba[kc_l[p, j]]
        n_dense = min(_PREFETCH, (T + _CHUNK - 1) // _CHUNK)
        for n in range(n_dense):
            t0 = n * _CHUNK
            tcn = min(_CHUNK, T - t0)
            d = taba[kc_l[:, t0:t0 + tcn]]  # [128, tcn, 128]
            im[f"slabd{n}"] = np.ascontiguousarray(
                d.reshape(BL, tcn * 128))
        in_maps.append(im)
    return in_maps


def kernel(corr, kc, problem, A, trans_logits, obs_logits_problem, init_logits,
           _T=None, _trace=False):
    T = _T or T_FULL
    nc = _emit_program(T, min(_CHUNK, T))
    in_maps = _prep_inputs(corr, kc, problem, A, trans_logits,
                           obs_logits_problem, init_logits, T)

    from concourse.bass_utils import run_bass_kernel_spmd
    res = run_bass_kernel_spmd(nc, in_maps, core_ids=list(range(N_CORES)),
                               trace=_trace)
    outs = [r["out"].reshape(BL, T, 2) for r in res.results]
    full = np.concatenate(outs, axis=0).astype(np.float32)
    kernel.last_results = res
    return full


if __name__ == "__main__":
    pass


# revision 30
# speedup vs baseline: 1.0475x; 1.0129x over previous
"""Trainium2 Bass kernel for the BKT (multi-HMM knowledge tracing) forward model.

Strategy: data-parallel over students (1024 students / 8 cores = 128 per core,
one SBUF partition per student). The T=500 time recursion runs locally per core.

Per-core algebra per step t:
    c      = A[kc[:,t]]                          [128,100]  (gathered, rows sum to 1)
    strm   = [w5_s0 | w5_s1] per (student,t)     [128,10]   (host-precomputed stream)
             where w5_s = [M4[s], M4[2+s], L4[s], L4[2+s], 0],
             M4 = (A @ log_t)[kc] + OLL(s') and L4 = log_obs[problem]
    a2_s   = sum_k c * alpha_s     (fused scalar_tensor_tensor accum; byproduct
                                    u_s = c * alpha_s is kept)
    e_s    = exp(strm_s + a2_s)                  (activation with bias=a2_s)
    ps5    = e_0 + e_1 = [se0,se1,po0,po1,q]
    lg5    = ln(ps5): a3 = lg5[:,0:2], log_py = lg5[:,2:4] - lg5[:,4:5]
    v_s    = alpha_s - u_s  (= alpha*(1-c), off the critical chain)
    alpha_s' = c * a3_s + v_s                    (one fused stt per s)

Only the A rows (indexed by kc, 512B each) are gathered on device; the small
per-step observation data (10 f32) is a pure function of the inputs and is
streamed in dense [128, T, 10] layout (loaded once into SBUF). Gathers are
spread across all 4 SWDGE queues with deep prefetch so descriptor generation
and DMA drain hide under the vector-engine recursion.

The predicted-output normalizer q = sum_o po_o collapses to e^{a2_0}+e^{a2_1}
because log_obs is normalized over o; sum_k c = 1 because A rows are a softmax.
"""

import os
from contextlib import ExitStack

import numpy as np

N_PROBLEMS = 10000
N_KCS = 100
BATCH = 1024
T_FULL = 500
N_CORES = 8
BL = BATCH // N_CORES  # 128 students per core

_CHUNK = 8  # time steps per gather slab (8*128 = 1024 = SWDGE ring capacity)
_PREFETCH = 5  # chunks of gather issued ahead of consumption
_NQ = 4  # SWDGE queues used round-robin
_BF16 = False  # keep alpha/c/u/v state in bf16 (DVE 2x mode); exp/ln stay fp32


def _log_softmax(x, axis):
    x = x.astype(np.float32)
    m = x.max(axis=axis, keepdims=True)
    e = np.exp(x - m)
    return (x - m) - np.log(e.sum(axis=axis, keepdims=True))


def _wrap_idx(flat):
    """dma_gather index layout: flat index i lives at partition i%16, col i//16,
    replicated across the 8 gpsimd cores (16-partition groups)."""
    assert flat.size % 16 == 0
    w = flat.astype(np.int16).reshape(-1, 16).T  # [16, N/16]
    return np.tile(w, (8, 1))  # [128, N/16]


def _host_tables(A, trans_logits, obs_logits_problem, init_logits):
    import ml_dtypes
    P = A.shape[0]
    K = trans_logits.shape[0]
    sdt = ml_dtypes.bfloat16 if _BF16 else np.float32
    log_t = _log_softmax(trans_logits, axis=1)  # [K,2,2] normalized over middle axis
    G = A.astype(np.float32) @ log_t.reshape(K, 4)  # [P,4] laid out (s,s')
    L = _log_softmax(obs_logits_problem, axis=2)  # [P,2,2] normalized over outputs

    taba = np.zeros((P, 128), sdt)
    taba[:, 0:100] = A.astype(sdt)

    la0 = _log_softmax(init_logits, axis=1)  # [K,2]
    alpha0 = np.empty((BL, 2 * K), sdt)
    alpha0[:, 0:K] = la0[:, 0].astype(sdt)  # s=0 block
    alpha0[:, K:] = la0[:, 1].astype(sdt)  # s=1 block
    return taba, G, L, alpha0


def _setup_act_tables():
    """Both Exp and Ln live in the 'natural_log_exp_and_others' ACT table
    set, but the default set ordering makes bacc pick a different set for
    each, inserting a ~2.7us ACT_TABLE_LOAD per activation (2 per time
    step!). Reorder the set list so that set comes first for both bacc's
    chooser and walrus (via BASS_ACT_ROOT_JSON_PATH), collapsing the loads
    to one for the whole kernel."""
    import glob
    import json
    import tempfile

    if os.environ.get("_BKT_ACT_TABLES"):
        return
    from neuronxcc.driver.Job import Job  # pyright: ignore[reportMissingImports]
    from neuronxcc.driver.jobs.support.FindActInfo import (  # pyright: ignore[reportMissingImports]
        findActInfoFile,
    )

    src = findActInfoFile(Job.getPackageDir(), "gen3")
    d = json.load(open(src))
    d["act_func_sets"] = sorted(
        d["act_func_sets"],
        key=lambda s: s["name"] != "natural_log_exp_and_others")
    tmp = tempfile.mkdtemp(prefix="bkt_act_")
    with open(tmp + "/act_info.json", "w") as f:
        json.dump(d, f)
    for p in glob.glob(os.path.dirname(src) + "/*"):
        b = os.path.basename(p)
        if b != "act_info.json":
            os.symlink(p, tmp + "/" + b)
    os.environ["BASS_ACT_ROOT_JSON_PATH"] = tmp + "/act_info.json"
    os.environ["_BKT_ACT_TABLES"] = "1"

    import concourse.bacc as bacc_mod
    import concourse.mybir as mybir

    def tables(arch):
        return {
            e["name"]: {mybir.ActivationFunctionType.from_pwp(v)
                        for v in e["act"].keys()}
            for e in d["act_func_sets"]
        }

    bacc_mod.get_activation_tables = tables


def _emit_program(T, Tc):
    import concourse.mybir as mybir
    import concourse.tile as tile
    from concourse import bacc

    _setup_act_tables()

    f32 = mybir.dt.float32
    sdt = mybir.dt.bfloat16 if _BF16 else f32
    i16 = mybir.dt.int16
    Alu = mybir.AluOpType
    Act = mybir.ActivationFunctionType
    K = N_KCS

    nc = bacc.Bacc("TRN2", target_bir_lowering=False, debug=False,
                   num_swdge_queues=_NQ)

    Th = min(2 * Tc, T)  # head steps staged in separate dense tensors
    n_dense = min(_PREFETCH, (T + Tc - 1) // Tc)  # host-pregathered chunks
    taba = nc.dram_tensor("taba", [N_PROBLEMS, 128], sdt, kind="ExternalInput")
    strm = nc.dram_tensor("strm", [BL, T * 10], f32, kind="ExternalInput")
    kcw = nc.dram_tensor("kcw", [128, T * 8], i16, kind="ExternalInput")
    strmh = nc.dram_tensor("strmh", [BL, Th * 10], f32, kind="ExternalInput")
    alpha0 = nc.dram_tensor("alpha0", [BL, 2 * K], sdt, kind="ExternalInput")
    slabd = [nc.dram_tensor(f"slabd{n}",
                            [128, min(Tc, T - n * Tc) * 128], sdt,
                            kind="ExternalInput")
             for n in range(n_dense)]
    out = nc.dram_tensor("out", [BL, T * 2], f32, kind="ExternalOutput")

    # SWDGE descriptor ring fits 1024 descriptors; each gathered row is one
    # descriptor, so cap each dma_gather call at 1024 indices (8 steps).
    assert Tc * 128 <= 1024
    chunks = []  # (t0, tcn)
    t0 = 0
    while t0 < T:
        chunks.append((t0, min(Tc, T - t0)))
        t0 += Tc
    n_chunks = len(chunks)

    from concourse import library_config

    with ExitStack() as ctx:
        tc = ctx.enter_context(tile.TileContext(nc))
        nc.gpsimd.load_library(library_config.mlp)
        idx_pool = ctx.enter_context(tc.tile_pool(name="idx", bufs=1))
        slab_pool = ctx.enter_context(
            tc.tile_pool(name="slabs", bufs=_PREFETCH + 1))
        state_pool = ctx.enter_context(tc.tile_pool(name="state", bufs=2))
        small_pool = ctx.enter_context(tc.tile_pool(name="small", bufs=4))
        u_pool = ctx.enter_context(tc.tile_pool(name="u", bufs=2))
        out_pool = ctx.enter_context(tc.tile_pool(name="outb", bufs=1))

        # DMA issue order = DMA queue drain order: the recursion's first
        # steps need alpha + strmh + the dense slabs, so those go first;
        # the bulk strm/kcw loads (needed from t=2*Tc / chunk _PREFETCH
        # onward) drain behind them.
        alpha = state_pool.tile([128, 2 * K], sdt, tag="alpha")
        nc.sync.dma_start(alpha[:], alpha0.ap())
        strmh_t = idx_pool.tile([128, Th * 10], f32, tag="strmh")
        nc.sync.dma_start(strmh_t[:], strmh.ap())
        kcw_t = idx_pool.tile([128, T * 8], i16, tag="kcw")
        strm_t = idx_pool.tile([128, T * 10], f32, tag="strm")

        outbuf = out_pool.tile([128, T * 2], f32)
        # per-step ln() results land here: [se0, se1, po0, po1, q] per t
        lgbuf = out_pool.tile([128, T * 5], f32)

        slabsA = [None] * n_chunks
        ni_regs = {}  # distinct chunk sizes get one register each
        for tcn in sorted({c[1] for c in chunks}):
            r = nc.gpsimd.alloc_register(f"ni{tcn}")
            nc.gpsimd.reg_mov(r, tcn * 128)
            ni_regs[tcn] = r

        def issue_gather(n):
            t0, tcn = chunks[n]
            sa = slab_pool.tile([128, Tc, 128], sdt, tag="slabA")
            if n < len(slabd):
                # first chunks come host-pregathered as dense DMAs so the
                # recursion starts without waiting for the gpsimd library
                # load + first dma_gather (~27us of startup otherwise)
                nc.sync.dma_start(sa[:, 0:tcn, :], slabd[n].ap())
            else:
                nc.gpsimd.dma_gather(
                    sa[:, 0:tcn, :], taba.ap(),
                    kcw_t[:, t0 * 8:(t0 + tcn) * 8],
                    num_idxs=tcn * 128, num_idxs_reg=ni_regs[tcn],
                    elem_size=128, queue_num=n % _NQ,
                )
            slabsA[n] = sa

        for n in range(min(_PREFETCH, n_chunks)):
            issue_gather(n)

        # bulk loads issue after the startup-critical DMAs above
        nc.sync.dma_start(strm_t[:], strm.ap())
        nc.sync.dma_start(kcw_t[:], kcw.ap())

        def c_ap(t):
            return slabsA[t // Tc][:, t % Tc, 0:K]

        def emit_a2(t, alpha_t):
            """a2_s = sum_k c * alpha_s, fused via scalar_tensor_tensor accum.
            alpha is blocked [128, (s k)]; slices pick each s. Returns
            (a2, u2) where u2 = c * alpha is reused for the state update."""
            a2 = small_pool.tile([128, 2], f32, tag="a2")
            u2 = u_pool.tile([128, 2 * K], sdt, tag="u2")
            for s in range(2):
                nc.vector.scalar_tensor_tensor(
                    out=u2[:, s * K:(s + 1) * K], in0=c_ap(t), scalar=0.0,
                    in1=alpha_t[:, s * K:(s + 1) * K],
                    op0=Alu.bypass, op1=Alu.mult,
                    accum_out=a2[:, s:s + 1],
                )
            return a2, u2

        # prologue: a2 for t=0
        a2, u2 = emit_a2(0, alpha)

        for n in range(n_chunks):
            if n + _PREFETCH < n_chunks:
                issue_gather(n + _PREFETCH)
            for j in range(chunks[n][1]):
                t = chunks[n][0] + j
                # e_s = exp(strm_s + a2_s); e_0 only waits on a2_0
                st = strmh_t if t < Th else strm_t
                e10 = small_pool.tile([128, 10], f32, tag="e10")
                for s in range(2):
                    nc.scalar.activation(
                        e10[:, 5 * s:5 * s + 5],
                        st[:, 10 * t + 5 * s:10 * t + 5 * s + 5],
                        Act.Exp, bias=a2[:, s:s + 1],
                    )
                # v = alpha*(1-c), off the Ln chain (runs under the Exps)
                v = u_pool.tile([128, 2 * K], sdt, tag="v")
                nc.vector.tensor_tensor(
                    out=v[:], in0=alpha[:], in1=u2[:], op=Alu.subtract,
                )
                ps5 = small_pool.tile([128, 5], f32, tag="ps5")
                nc.vector.tensor_tensor(
                    out=ps5[:], in0=e10[:, 0:5], in1=e10[:, 5:10], op=Alu.add,
                )
                lg5 = lgbuf[:, 5 * t:5 * t + 5]
                nc.scalar.activation(lg5, ps5[:], Act.Ln)
                # state update per s: alpha_s' = c*a3_s + v_s, then a2 for
                # s before moving to the other s so Exp_0 can start early
                alpha_new = state_pool.tile([128, 2 * K], sdt, tag="alpha")
                last = t + 1 >= T
                if not last:
                    a2n = small_pool.tile([128, 2], f32, tag="a2")
                    u2n = u_pool.tile([128, 2 * K], sdt, tag="u2")
                for s in range(2):
                    nc.vector.scalar_tensor_tensor(
                        out=alpha_new[:, s * K:(s + 1) * K], in0=c_ap(t),
                        scalar=lgbuf[:, 5 * t + s:5 * t + s + 1],
                        in1=v[:, s * K:(s + 1) * K],
                        op0=Alu.mult, op1=Alu.add,
                    )
                    if not last:
                        nc.vector.scalar_tensor_tensor(
                            out=u2n[:, s * K:(s + 1) * K], in0=c_ap(t + 1),
                            scalar=0.0,
                            in1=alpha_new[:, s * K:(s + 1) * K],
                            op0=Alu.bypass, op1=Alu.mult,
                            accum_out=a2n[:, s:s + 1],
                        )
                alpha = alpha_new
                if not last:
                    a2, u2 = a2n, u2n

        # normalize all outputs at once: log_py[t, o] = lpo[t, o] - lq[t]
        lg3 = lgbuf[:].rearrange("p (t f) -> p t f", f=5)
        nc.vector.tensor_tensor(
            out=outbuf[:].rearrange("p (t o) -> p t o", o=2),
            in0=lg3[:, :, 2:4],
            in1=lg3[:, :, 4:5].broadcast_to([128, T, 2]),
            op=Alu.subtract,
        )
        nc.sync.dma_start(out.ap(), outbuf[:])

    nc.compile()
    return nc


def _prep_inputs(corr, kc, problem, A, trans_logits, obs_logits_problem, init_logits, T):
    corr = np.asarray(corr).astype(np.int64)
    kc = np.asarray(kc).astype(np.int64)
    problem = np.asarray(problem).astype(np.int64)
    taba, G, L, alpha0 = _host_tables(
        np.asarray(A), np.asarray(trans_logits),
        np.asarray(obs_logits_problem), np.asarray(init_logits))

    in_maps = []
    for i in range(N_CORES):
        sl = slice(i * BL, (i + 1) * BL)
        kc_l = kc[sl, :T]  # [128, T]
        pp_l = problem[sl, :T]
        cr_l = corr[sl, :T]
        # dense per-(student,step) stream, grouped by state s:
        # strm[.., 5s:5s+5] = [M4[s], M4[2+s], L4[s], L4[2+s], 0]
        Gk = G[kc_l]  # [128,T,4]
        Lp = L[pp_l]  # [128,T,2,2]
        OLL = np.take_along_axis(
            Lp, cr_l[:, :, None, None], axis=3)[:, :, :, 0]  # [128,T,2]
        M4 = (Gk.reshape(BL, T, 2, 2) + OLL[:, :, None, :]).reshape(BL, T, 4)
        L4 = Lp.transpose(0, 1, 3, 2).reshape(BL, T, 4)  # L4[2o+s] = L[s,o]
        Z = np.zeros((BL, T), np.float32)
        strm = np.stack(
            [M4[..., 0], M4[..., 2], L4[..., 0], L4[..., 2], Z,
             M4[..., 1], M4[..., 3], L4[..., 1], L4[..., 3], Z],
            axis=2).astype(np.float32).reshape(BL, T * 10)
        # gather flat order: i = j*128 + p  ->  idx = kc_l[p, j]
        kcw = _wrap_idx(kc_l.T.ravel())
        Th = min(2 * _CHUNK, T)
        im = {
            "taba": taba, "strm": strm, "kcw": kcw, "alpha0": alpha0,
            "strmh": np.ascontiguousarray(strm[:, 0:Th * 10]),
        }
        # host-pregathered A slabs for the first _PREFETCH chunks: matches
        # the dma_gather output layout slab[p, j, :] = taba[kc_l[p, j]]
        n_dense = min(_PREFETCH, (T + _CHUNK - 1) // _CHUNK)
        for n in range(n_dense):
            t0 = n * _CHUNK
            tcn = min(_CHUNK, T - t0)
            d = taba[kc_l[:, t0:t0 + tcn]]  # [128, tcn, 128]
            im[f"slabd{n}"] = np.ascontiguousarray(
                d.reshape(BL, tcn * 128))
        in_maps.append(im)
    return in_maps


def kernel(corr, kc, problem, A, trans_logits, obs_logits_problem, init_logits,
           _T=None, _trace=False):
    T = _T or T_FULL
    nc = _emit_program(T, min(_CHUNK, T))
    in_maps = _prep_inputs(corr, kc, problem, A, trans_logits,
                           obs_logits_problem, init_logits, T)

    from concourse.bass_utils import run_bass_kernel_spmd
    res = run_bass_kernel_spmd(nc, in_maps, core_ids=list(range(N_CORES)),
                               trace=_trace)
    outs = [r["out"].reshape(BL, T, 2) for r in res.results]
    full = np.concatenate(outs, axis=0).astype(np.float32)
    kernel.last_results = res
    return full


if __name__ == "__main__":
    pass


# revision 35
# speedup vs baseline: 1.0494x; 1.0018x over previous
"""Trainium2 Bass kernel for the BKT (multi-HMM knowledge tracing) forward model.

Strategy: data-parallel over students (1024 students / 8 cores = 128 per core,
one SBUF partition per student). The T=500 time recursion runs locally per core.

Per-core algebra per step t:
    c      = A[kc[:,t]]                          [128,100]  (gathered, rows sum to 1)
    strm   = [w5_s0 | w5_s1] per (student,t)     [128,10]   (host-precomputed stream)
             where w5_s = [M4[s], M4[2+s], L4[s], L4[2+s], 0],
             M4 = (A @ log_t)[kc] + OLL(s') and L4 = log_obs[problem]
    a2_s   = sum_k c * alpha_s     (fused scalar_tensor_tensor accum; byproduct
                                    u_s = c * alpha_s is kept)
    e_s    = exp(strm_s + a2_s)                  (activation with bias=a2_s)
    ps5    = e_0 + e_1 = [se0,se1,po0,po1,q]
    lg5    = ln(ps5): a3 = lg5[:,0:2], log_py = lg5[:,2:4] - lg5[:,4:5]
    v_s    = alpha_s - u_s  (= alpha*(1-c), off the critical chain)
    alpha_s' = c * a3_s + v_s                    (one fused stt per s)

Only the A rows (indexed by kc, 512B each) are gathered on device; the small
per-step observation data (10 f32) is a pure function of the inputs and is
streamed in dense [128, T, 10] layout (loaded once into SBUF). Gathers are
spread across all 4 SWDGE queues with deep prefetch so descriptor generation
and DMA drain hide under the vector-engine recursion.

The predicted-output normalizer q = sum_o po_o collapses to e^{a2_0}+e^{a2_1}
because log_obs is normalized over o; sum_k c = 1 because A rows are a softmax.
"""

import os
from contextlib import ExitStack

import numpy as np

N_PROBLEMS = 10000
N_KCS = 100
BATCH = 1024
T_FULL = 500
N_CORES = 8
BL = BATCH // N_CORES  # 128 students per core

_CHUNK = 8  # time steps per gather slab (8*128 = 1024 = SWDGE ring capacity)
_PREFETCH = 5  # chunks of gather issued ahead of consumption
_NQ = 4  # SWDGE queues used round-robin
_BF16 = False  # keep alpha/c/u/v state in bf16 (DVE 2x mode); exp/ln stay fp32


def _log_softmax(x, axis):
    x = x.astype(np.float32)
    m = x.max(axis=axis, keepdims=True)
    e = np.exp(x - m)
    return (x - m) - np.log(e.sum(axis=axis, keepdims=True))


def _wrap_idx(flat):
    """dma_gather index layout: flat index i lives at partition i%16, col i//16,
    replicated across the 8 gpsimd cores (16-partition groups)."""
    assert flat.size % 16 == 0
    w = flat.astype(np.int16).reshape(-1, 16).T  # [16, N/16]
    return np.tile(w, (8, 1))  # [128, N/16]


def _host_tables(A, trans_logits, obs_logits_problem, init_logits):
    import ml_dtypes
    P = A.shape[0]
    K = trans_logits.shape[0]
    sdt = ml_dtypes.bfloat16 if _BF16 else np.float32
    log_t = _log_softmax(trans_logits, axis=1)  # [K,2,2] normalized over middle axis
    G = A.astype(np.float32) @ log_t.reshape(K, 4)  # [P,4] laid out (s,s')
    L = _log_softmax(obs_logits_problem, axis=2)  # [P,2,2] normalized over outputs

    taba = np.zeros((P, 128), sdt)
    taba[:, 0:100] = A.astype(sdt)

    la0 = _log_softmax(init_logits, axis=1)  # [K,2]
    alpha0 = np.empty((BL, 2 * K), sdt)
    alpha0[:, 0:K] = la0[:, 0].astype(sdt)  # s=0 block
    alpha0[:, K:] = la0[:, 1].astype(sdt)  # s=1 block
    return taba, G, L, alpha0


def _setup_act_tables():
    """Both Exp and Ln live in the 'natural_log_exp_and_others' ACT table
    set, but the default set ordering makes bacc pick a different set for
    each, inserting a ~2.7us ACT_TABLE_LOAD per activation (2 per time
    step!). Reorder the set list so that set comes first for both bacc's
    chooser and walrus (via BASS_ACT_ROOT_JSON_PATH), collapsing the loads
    to one for the whole kernel."""
    import glob
    import json
    import tempfile

    if os.environ.get("_BKT_ACT_TABLES"):
        return
    from neuronxcc.driver.Job import Job  # pyright: ignore[reportMissingImports]
    from neuronxcc.driver.jobs.support.FindActInfo import (  # pyright: ignore[reportMissingImports]
        findActInfoFile,
    )

    src = findActInfoFile(Job.getPackageDir(), "gen3")
    d = json.load(open(src))
    d["act_func_sets"] = sorted(
        d["act_func_sets"],
        key=lambda s: s["name"] != "natural_log_exp_and_others")
    tmp = tempfile.mkdtemp(prefix="bkt_act_")
    with open(tmp + "/act_info.json", "w") as f:
        json.dump(d, f)
    for p in glob.glob(os.path.dirname(src) + "/*"):
        b = os.path.basename(p)
        if b != "act_info.json":
            os.symlink(p, tmp + "/" + b)
    os.environ["BASS_ACT_ROOT_JSON_PATH"] = tmp + "/act_info.json"
    os.environ["_BKT_ACT_TABLES"] = "1"

    import concourse.bacc as bacc_mod
    import concourse.mybir as mybir

    def tables(arch):
        return {
            e["name"]: {mybir.ActivationFunctionType.from_pwp(v)
                        for v in e["act"].keys()}
            for e in d["act_func_sets"]
        }

    bacc_mod.get_activation_tables = tables


def _emit_program(T, Tc):
    import concourse.mybir as mybir
    import concourse.tile as tile
    from concourse import bacc

    _setup_act_tables()

    f32 = mybir.dt.float32
    sdt = mybir.dt.bfloat16 if _BF16 else f32
    i16 = mybir.dt.int16
    Alu = mybir.AluOpType
    Act = mybir.ActivationFunctionType
    K = N_KCS

    nc = bacc.Bacc("TRN2", target_bir_lowering=False, debug=False,
                   num_swdge_queues=_NQ)

    Th = min(2 * Tc, T)  # head steps staged in separate dense tensors
    n_dense = min(_PREFETCH, (T + Tc - 1) // Tc)  # host-pregathered chunks
    taba = nc.dram_tensor("taba", [N_PROBLEMS, 128], sdt, kind="ExternalInput")
    strm = nc.dram_tensor("strm", [BL, T * 10], f32, kind="ExternalInput")
    kcw = nc.dram_tensor("kcw", [128, T * 8], i16, kind="ExternalInput")
    strmh = nc.dram_tensor("strmh", [BL, Th * 10], f32, kind="ExternalInput")
    alpha0 = nc.dram_tensor("alpha0", [BL, 2 * K], sdt, kind="ExternalInput")
    # 2-step head of chunk 0 as its own tensor: lands first so step 0's a2
    # doesn't wait for the whole first slab
    Td = min(2, T)
    def _sd_w(n):
        tcn = min(Tc, T - n * Tc)
        return tcn - Td if (n == 0 and tcn > Td) else tcn

    slabd = [nc.dram_tensor(f"slabd{n}", [128, _sd_w(n) * 128], sdt,
                            kind="ExternalInput")
             for n in range(n_dense)]
    slabd0h = nc.dram_tensor("slabd0h", [128, Td * 128], sdt,
                             kind="ExternalInput")
    out = nc.dram_tensor("out", [BL, T * 2], f32, kind="ExternalOutput")

    # SWDGE descriptor ring fits 1024 descriptors; each gathered row is one
    # descriptor, so cap each dma_gather call at 1024 indices (8 steps).
    assert Tc * 128 <= 1024
    chunks = []  # (t0, tcn)
    t0 = 0
    while t0 < T:
        chunks.append((t0, min(Tc, T - t0)))
        t0 += Tc
    n_chunks = len(chunks)

    from concourse import library_config

    with ExitStack() as ctx:
        tc = ctx.enter_context(tile.TileContext(nc))
        nc.gpsimd.load_library(library_config.mlp)
        idx_pool = ctx.enter_context(tc.tile_pool(name="idx", bufs=1))
        slab_pool = ctx.enter_context(
            tc.tile_pool(name="slabs", bufs=_PREFETCH + 1))
        state_pool = ctx.enter_context(tc.tile_pool(name="state", bufs=2))
        small_pool = ctx.enter_context(tc.tile_pool(name="small", bufs=4))
        u_pool = ctx.enter_context(tc.tile_pool(name="u", bufs=2))
        out_pool = ctx.enter_context(tc.tile_pool(name="outb", bufs=1))

        # DMA issue order = DMA queue drain order: the recursion's first
        # steps need alpha + strmh + the dense slabs, so those go first;
        # the bulk strm/kcw loads (needed from t=2*Tc / chunk _PREFETCH
        # onward) drain behind them.
        alpha = state_pool.tile([128, 2 * K], sdt, tag="alpha")
        nc.sync.dma_start(alpha[:], alpha0.ap())
        strmh_t = idx_pool.tile([128, Th * 10], f32, tag="strmh")
        nc.sync.dma_start(strmh_t[:], strmh.ap())
        kcw_t = idx_pool.tile([128, T * 8], i16, tag="kcw")
        strm_t = idx_pool.tile([128, T * 10], f32, tag="strm")

        outbuf = out_pool.tile([128, T * 2], f32)
        # per-step ln() results land here: [se0, se1, po0, po1, q] per t
        lgbuf = out_pool.tile([128, T * 5], f32)

        slabsA = [None] * n_chunks
        ni_regs = {}  # distinct chunk sizes get one register each
        for tcn in sorted({c[1] for c in chunks}):
            r = nc.gpsimd.alloc_register(f"ni{tcn}")
            nc.gpsimd.reg_mov(r, tcn * 128)
            ni_regs[tcn] = r

        def issue_gather(n):
            t0, tcn = chunks[n]
            sa = slab_pool.tile([128, Tc, 128], sdt, tag="slabA")
            if n < len(slabd):
                # first chunks come host-pregathered as dense DMAs so the
                # recursion starts without waiting for the gpsimd library
                # load + first dma_gather (~27us of startup otherwise)
                if n == 0 and Td < tcn:
                    # chunk 0 split: 2-step head first, rest behind it
                    nc.sync.dma_start(sa[:, 0:Td, :], slabd0h.ap())
                    nc.sync.dma_start(sa[:, Td:tcn, :], slabd[n].ap())
                else:
                    nc.sync.dma_start(sa[:, 0:tcn, :], slabd[n].ap())
            else:
                nc.gpsimd.dma_gather(
                    sa[:, 0:tcn, :], taba.ap(),
                    kcw_t[:, t0 * 8:(t0 + tcn) * 8],
                    num_idxs=tcn * 128, num_idxs_reg=ni_regs[tcn],
                    elem_size=128, queue_num=n % _NQ,
                )
            slabsA[n] = sa

        for n in range(min(_PREFETCH, n_chunks)):
            issue_gather(n)

        # bulk loads issue after the startup-critical DMAs above
        nc.sync.dma_start(strm_t[:], strm.ap())
        nc.sync.dma_start(kcw_t[:], kcw.ap())

        def c_ap(t):
            return slabsA[t // Tc][:, t % Tc, 0:K]

        def emit_a2(t, alpha_t):
            """a2_s = sum_k c * alpha_s, fused via scalar_tensor_tensor accum.
            alpha is blocked [128, (s k)]; slices pick each s. Returns
            (a2, u2) where u2 = c * alpha is reused for the state update."""
            a2 = small_pool.tile([128, 2], f32, tag="a2")
            u2 = u_pool.tile([128, 2 * K], sdt, tag="u2")
            for s in range(2):
                nc.vector.scalar_tensor_tensor(
                    out=u2[:, s * K:(s + 1) * K], in0=c_ap(t), scalar=0.0,
                    in1=alpha_t[:, s * K:(s + 1) * K],
                    op0=Alu.bypass, op1=Alu.mult,
                    accum_out=a2[:, s:s + 1],
                )
            return a2, u2

        # prologue: a2 for t=0
        a2, u2 = emit_a2(0, alpha)

        for n in range(n_chunks):
            if n + _PREFETCH < n_chunks:
                issue_gather(n + _PREFETCH)
            for j in range(chunks[n][1]):
                t = chunks[n][0] + j
                # e_s = exp(strm_s + a2_s); e_0 only waits on a2_0
                st = strmh_t if t < Th else strm_t
                e10 = small_pool.tile([128, 10], f32, tag="e10")
                for s in range(2):
                    nc.scalar.activation(
                        e10[:, 5 * s:5 * s + 5],
                        st[:, 10 * t + 5 * s:10 * t + 5 * s + 5],
                        Act.Exp, bias=a2[:, s:s + 1],
                    )
                # v = alpha*(1-c), off the Ln chain (runs under the Exps)
                v = u_pool.tile([128, 2 * K], sdt, tag="v")
                nc.vector.tensor_tensor(
                    out=v[:], in0=alpha[:], in1=u2[:], op=Alu.subtract,
                )
                ps5 = small_pool.tile([128, 5], f32, tag="ps5")
                nc.vector.tensor_tensor(
                    out=ps5[:], in0=e10[:, 0:5], in1=e10[:, 5:10], op=Alu.add,
                )
                lg5 = lgbuf[:, 5 * t:5 * t + 5]
                nc.scalar.activation(lg5, ps5[:], Act.Ln)
                # state update per s: alpha_s' = c*a3_s + v_s, then a2 for
                # s before moving to the other s so Exp_0 can start early
                alpha_new = state_pool.tile([128, 2 * K], sdt, tag="alpha")
                last = t + 1 >= T
                if not last:
                    a2n = small_pool.tile([128, 2], f32, tag="a2")
                    u2n = u_pool.tile([128, 2 * K], sdt, tag="u2")
                for s in range(2):
                    nc.vector.scalar_tensor_tensor(
                        out=alpha_new[:, s * K:(s + 1) * K], in0=c_ap(t),
                        scalar=lgbuf[:, 5 * t + s:5 * t + s + 1],
                        in1=v[:, s * K:(s + 1) * K],
                        op0=Alu.mult, op1=Alu.add,
                    )
                    if not last:
                        nc.vector.scalar_tensor_tensor(
                            out=u2n[:, s * K:(s + 1) * K], in0=c_ap(t + 1),
                            scalar=0.0,
                            in1=alpha_new[:, s * K:(s + 1) * K],
                            op0=Alu.bypass, op1=Alu.mult,
                            accum_out=a2n[:, s:s + 1],
                        )
                alpha = alpha_new
                if not last:
                    a2, u2 = a2n, u2n

        # normalize all outputs at once: log_py[t, o] = lpo[t, o] - lq[t]
        lg3 = lgbuf[:].rearrange("p (t f) -> p t f", f=5)
        nc.vector.tensor_tensor(
            out=outbuf[:].rearrange("p (t o) -> p t o", o=2),
            in0=lg3[:, :, 2:4],
            in1=lg3[:, :, 4:5].broadcast_to([128, T, 2]),
            op=Alu.subtract,
        )
        nc.sync.dma_start(out.ap(), outbuf[:])

    nc.compile()
    return nc


def _prep_inputs(corr, kc, problem, A, trans_logits, obs_logits_problem, init_logits, T):
    corr = np.asarray(corr).astype(np.int64)
    kc = np.asarray(kc).astype(np.int64)
    problem = np.asarray(problem).astype(np.int64)
    taba, G, L, alpha0 = _host_tables(
        np.asarray(A), np.asarray(trans_logits),
        np.asarray(obs_logits_problem), np.asarray(init_logits))

    in_maps = []
    for i in range(N_CORES):
        sl = slice(i * BL, (i + 1) * BL)
        kc_l = kc[sl, :T]  # [128, T]
        pp_l = problem[sl, :T]
        cr_l = corr[sl, :T]
        # dense per-(student,step) stream, grouped by state s:
        # strm[.., 5s:5s+5] = [M4[s], M4[2+s], L4[s], L4[2+s], 0]
        Gk = G[kc_l]  # [128,T,4]
        Lp = L[pp_l]  # [128,T,2,2]
        OLL = np.take_along_axis(
            Lp, cr_l[:, :, None, None], axis=3)[:, :, :, 0]  # [128,T,2]
        M4 = (Gk.reshape(BL, T, 2, 2) + OLL[:, :, None, :]).reshape(BL, T, 4)
        L4 = Lp.transpose(0, 1, 3, 2).reshape(BL, T, 4)  # L4[2o+s] = L[s,o]
        Z = np.zeros((BL, T), np.float32)
        strm = np.stack(
            [M4[..., 0], M4[..., 2], L4[..., 0], L4[..., 2], Z,
             M4[..., 1], M4[..., 3], L4[..., 1], L4[..., 3], Z],
            axis=2).astype(np.float32).reshape(BL, T * 10)
        # gather flat order: i = j*128 + p  ->  idx = kc_l[p, j]
        kcw = _wrap_idx(kc_l.T.ravel())
        Th = min(2 * _CHUNK, T)
        im = {
            "taba": taba, "strm": strm, "kcw": kcw, "alpha0": alpha0,
            "strmh": np.ascontiguousarray(strm[:, 0:Th * 10]),
        }
        # host-pregathered A slabs for the first _PREFETCH chunks: matches
        # the dma_gather output layout slab[p, j, :] = taba[kc_l[p, j]]
        n_dense = min(_PREFETCH, (T + _CHUNK - 1) // _CHUNK)
        Td = min(2, T)
        for n in range(n_dense):
            t0 = n * _CHUNK
            tcn = min(_CHUNK, T - t0)
            d = taba[kc_l[:, t0:t0 + tcn]]  # [128, tcn, 128]
            if n == 0 and tcn > Td:
                im["slabd0h"] = np.ascontiguousarray(
                    d[:, 0:Td].reshape(BL, Td * 128))
                d = d[:, Td:]
                tcn -= Td
            elif n == 0:
                im["slabd0h"] = np.ascontiguousarray(
                    d.reshape(BL, tcn * 128))
            im[f"slabd{n}"] = np.ascontiguousarray(
                d.reshape(BL, tcn * 128))
        in_maps.append(im)
    return in_maps


def kernel(corr, kc, problem, A, trans_logits, obs_logits_problem, init_logits,
           _T=None, _trace=False):
    T = _T or T_FULL
    nc = _emit_program(T, min(_CHUNK, T))
    in_maps = _prep_inputs(corr, kc, problem, A, trans_logits,
                           obs_logits_problem, init_logits, T)

    from concourse.bass_utils import run_bass_kernel_spmd
    res = run_bass_kernel_spmd(nc, in_maps, core_ids=list(range(N_CORES)),
                               trace=_trace)
    outs = [r["out"].reshape(BL, T, 2) for r in res.results]
    full = np.concatenate(outs, axis=0).astype(np.float32)
    kernel.last_results = res
    return full


if __name__ == "__main__":
    pass
